# revision 41
# baseline (speedup 1.0000x reference)
"""Trainium2 Bass kernel for nn_AttentiveEncoderPOS (embed+concat+linear+self-attention).

Strategy (8 cores, sequence-parallel with AllGather):
  - Each core gathers/computes only ITS 1024-row slice of
    L = concat(emb[ids], pos[pids]) @ W.T + b, in transposed layout
    (L.T, h on partitions), quantized to fp8 (x32 scale). A per-chunk
    8-core AllGather (fp8 payload) shares all slices while compute runs.
  - Full fp8 L.T stays resident in SBUF in DoubleRow layout [128, 2, N]
    (adjacent h-tile pairs stacked), so score matmuls run fp8 DoubleRow
    (2 contraction rows/cycle). V tiles are transposed out of the same
    resident fp8 L.T on the PE and widened to bf16; exp() output is bf16,
    so attn @ V runs bf16. Phase 2 streams nothing from DRAM.
  - Scores are tiny (|s|<0.025) so exp() without max-subtraction is exact
    softmax; denominator accumulates on the vector engine.
"""

import numpy as np

import concourse.bass as bass
import concourse.mybir as mybir
from concourse import bacc
from concourse.tile import TileContext
from concourse.bass_utils import run_bass_kernel_spmd
from concourse.masks import make_identity

N = 8192
H = 1024
VOCAB = 50257
POS = 64
NCORES = 8
NL = N // NCORES          # 1024 rows (queries) per core
P = 128
HT = H // P               # 8 h tiles
HT2 = HT // 2             # 4 DoubleRow h-pair tiles
K2 = 2 * H
KTI = K2 // P             # 16 contraction tiles for the linear
RTOT = NL // P            # 8 row tiles per core
CHUNK = 512
NCH = NL // CHUNK         # 2 phase-1 chunks
RT = CHUNK // P           # 4 row tiles / chunk
KT = N // P               # 64 key tiles
BLK = 8                   # key tiles per phase-2 block
NBLK = KT // BLK
QTP = NL // P             # 8 q tiles
QH = NL // CHUNK          # 2 score chunks along q
HH = H // CHUNK           # 2 A@V output chunks
FSCALE = 32.0             # fp8 quantization scale for L
KE = 8.0                  # fp8 scale for e' = KE*(exp(s)-1)
SCALE = 1.0 / 32.0        # 1/sqrt(H)
SCALE8 = SCALE / (FSCALE * FSCALE)

BF = mybir.dt.bfloat16
F8 = mybir.dt.float8e4
F32 = mybir.dt.float32
I32 = mybir.dt.int32
EXP = mybir.ActivationFunctionType.Exp
COPY = mybir.ActivationFunctionType.Copy
DR = mybir.MatmulPerfMode.DoubleRow


def build_nc():
    nc = bacc.Bacc()
    ids = nc.declare_dram_parameter("ids", [RTOT, P, 1], I32, isOutput=False)
    pids = nc.declare_dram_parameter("pids", [RTOT, P, 1], I32, isOutput=False)
    # emb/pemb/wt arrive pre-quantized to fp8 (x32) from the host
    emb = nc.declare_dram_parameter("emb", [VOCAB, H], F8, isOutput=False)
    pemb = nc.declare_dram_parameter("pemb", [POS, H], F8, isOutput=False)
    wt = nc.declare_dram_parameter("wt", [K2, H], F8, isOutput=False)  # 32*W.T
    bias = nc.declare_dram_parameter("bias", [HT, P, 1], F32, isOutput=False)
    out = nc.declare_dram_parameter("out", [NL, H], F32, isOutput=True)

    # AllGather bounce buffers (fp8), one pair per 512-row phase-1 chunk so
    # the gather of chunk 0 overlaps phase-1 compute of chunk 1 and phase 2.
    # L.T tiles and V-natural row tiles travel in separate collectives so
    # score matmuls only wait on the L gather.
    ag_in = [nc.dram_tensor(f"ag_in{c}", [HT, P, CHUNK], F8) for c in range(NCH)]
    ag_out = [
        nc.dram_tensor(
            f"ag_out{c}", [NCORES, HT, P, CHUNK], F8, addr_space="Shared"
        )
        for c in range(NCH)
    ]
    agv_in = [nc.dram_tensor(f"agv_in{c}", [RT, P, H], F8) for c in range(NCH)]
    agv_out = [
        nc.dram_tensor(
            f"agv_out{c}", [NCORES, RT, P, H], F8, addr_space="Shared"
        )
        for c in range(NCH)
    ]
    # AllReduce for the global column-sum of 32*L (the attn@V fp8 correction)
    ar_in = nc.dram_tensor("ar_in", [P, HT], F32)
    ar_out = nc.dram_tensor("ar_out", [P, HT], F32, addr_space="Shared")

    with TileContext(nc) as tc:
        with (
            tc.tile_pool(name="const", bufs=1) as const,
            tc.tile_pool(name="qres", bufs=1) as qresp,
        ):
            ident8 = const.tile([P, P], F8)
            make_identity(nc, ident8[:])
            ident32 = const.tile([P, P], F32)
            make_identity(nc, ident32[:])
            ones32 = const.tile([P, 1], F32)
            nc.gpsimd.memset(ones32[:], 1.0)
            kerow32 = const.tile([1, P], F32)
            nc.gpsimd.memset(kerow32[:], KE)
            one1 = const.tile([1, 1], F32)
            nc.gpsimd.memset(one1[:], 1.0 / (FSCALE * KE))
            b_sb = const.tile([P, HT], F32)
            nc.sync.dma_start(
                out=b_sb[:].rearrange("p (h u) -> p h u", h=HT),
                in_=bias.rearrange("h p u -> p h u"),
            )
            # the fp8 linear computes 1024*(X@W.T); fold bias pre-scaled
            b1024_sb = const.tile([P, HT], F32)
            nc.vector.tensor_scalar_mul(
                out=b1024_sb[:], in0=b_sb[:], scalar1=FSCALE * FSCALE
            )
            # own fp8 L.T chunk in DoubleRow layout (these are the queries)
            q8 = [
                qresp.tile([P, 2, NL], F8, tag=f"q{h2}", name=f"q{h2}")
                for h2 in range(HT2)
            ]

            # ---------------- Phase 1: own L.T chunk ----------------
            with (
                tc.tile_pool(name="wtp", bufs=1) as wtp,
                tc.tile_pool(name="idp", bufs=8) as idp,
                tc.tile_pool(name="xfp", bufs=RTOT + 1) as xfp,
                tc.tile_pool(name="xbp", bufs=RT + 1) as xbp,
                tc.tile_pool(name="xtp", bufs=KTI + 2) as xtp,
                tc.tile_pool(name="tps", bufs=2, space="PSUM") as tps,
                tc.tile_pool(name="mps", bufs=2, space="PSUM") as mps,
            ):
                # ids first, then chunk-0 gathers, so the W loads (on the
                # scalar queue) don't gate the first AllGather.
                idts, pidts = [], []
                for t in range(RTOT):
                    idt = idp.tile([P, 1], I32, tag="id")
                    nc.sync.dma_start(out=idt[:], in_=ids[t])
                    pidt = idp.tile([P, 1], I32, tag="pid")
                    nc.sync.dma_start(out=pidt[:], in_=pids[t])
                    idts.append(idt)
                    pidts.append(pidt)

                # gathers for all row tiles (fp8 tables) issue first, then W
                xfs = []
                for t in range(RTOT):
                    xf = xfp.tile([P, K2], F8, tag="xf")
                    nc.gpsimd.indirect_dma_start(
                        out=xf[:, 0:H],
                        out_offset=None,
                        in_=emb[:],
                        in_offset=bass.IndirectOffsetOnAxis(
                            ap=idts[t][:, :1], axis=0
                        ),
                    )
                    nc.gpsimd.indirect_dma_start(
                        out=xf[:, H:K2],
                        out_offset=None,
                        in_=pemb[:],
                        in_offset=bass.IndirectOffsetOnAxis(
                            ap=pidts[t][:, :1], axis=0
                        ),
                    )
                    xfs.append(xf)

                # 32*W.T, fp8, straight into DoubleRow layout
                w8 = []
                for k in range(KTI):
                    if k % 2 == 0:
                        w8.append(
                            wtp.tile([P, 2, H], F8, tag=f"w8_{k//2}", name=f"w8_{k//2}")
                        )
                    nc.scalar.dma_start(
                        out=w8[k // 2][:, k % 2, :], in_=wt[k * P : (k + 1) * P, :]
                    )

                for ch in range(NCH):
                    x8bs = [xfs[ch * RT + rt] for rt in range(RT)]
                    x8ts = []
                    for k2 in range(KTI // 2):
                        x8t = xtp.tile([P, 2, CHUNK], F8, tag="xt")
                        for r in range(2):
                            pt = tps.tile([P, CHUNK, 2], F8, tag="tp")
                            k = 2 * k2 + r
                            for rt in range(RT):
                                nc.tensor.transpose(
                                    pt[:, rt * P : (rt + 1) * P, 0],
                                    x8bs[rt][:, k * P : (k + 1) * P],
                                    ident8[:],
                                )
                            nc.scalar.activation(
                                out=x8t[:, r, :], in_=pt[:, :, 0], func=COPY
                            )
                        x8ts.append(x8t)

                    # linear (fp8 DR): 1024*L.T[ht, chunk]
                    csl = slice(ch * CHUNK, (ch + 1) * CHUNK)
                    for ht in range(HT):
                        pm = mps.tile([P, CHUNK], F32, tag="mp")
                        for k2 in range(KTI // 2):
                            nc.tensor.matmul(
                                pm[:],
                                lhsT=w8[k2][:, :, ht * P : (ht + 1) * P],
                                rhs=x8ts[k2][:],
                                start=(k2 == 0),
                                stop=(k2 == KTI // 2 - 1),
                                perf_mode=DR,
                            )
                        # fp8 quantize: q8 = 32*(pm/1024 + b) = (pm + 1024b)/32
                        nc.vector.tensor_scalar(
                            out=q8[ht // 2][:, ht % 2, csl],
                            in0=pm[:],
                            scalar1=b1024_sb[:, ht : ht + 1],
                            scalar2=1.0 / FSCALE,
                            op0=mybir.AluOpType.add,
                            op1=mybir.AluOpType.mult,
                        )
                        nc.sync.dma_start(
                            out=ag_in[ch][ht], in_=q8[ht // 2][:, ht % 2, csl]
                        )
                    # AllGather the L.T chunk first: scores only need this one
                    nc.gpsimd.collective_compute(
                        "AllGather",
                        mybir.AluOpType.bypass,
                        replica_groups=[list(range(NCORES))],
                        ins=[ag_in[ch][:].opt()],
                        outs=[ag_out[ch][:].opt()],
                    )
                    # V-natural tiles for this chunk (so phase 2 needn't
                    # transpose): transpose own L.T rows back to [keys, h]
                    for rt in range(RT):
                        rsl = slice(ch * CHUNK + rt * P, ch * CHUNK + (rt + 1) * P)
                        ptv = tps.tile([P, H, 2], F8, tag="tpv")
                        for ht in range(HT):
                            nc.tensor.transpose(
                                ptv[:, ht * P : (ht + 1) * P, 0],
                                q8[ht // 2][:, ht % 2, rsl],
                                ident8[:],
                            )
                        vn = xbp.tile([P, H], F8, tag="vn", bufs=3)
                        nc.scalar.activation(
                            out=vn[:], in_=ptv[:, :, 0], func=COPY
                        )
                        nc.sync.dma_start(out=agv_in[ch][rt], in_=vn[:])
                    nc.gpsimd.collective_compute(
                        "AllGather",
                        mybir.AluOpType.bypass,
                        replica_groups=[list(range(NCORES))],
                        ins=[agv_in[ch][:].opt()],
                        outs=[agv_out[ch][:].opt()],
                    )

                # own-chunk column sums of 32*L (for the attn@V correction):
                # sum q8 over keys on the scalar engine, AllReduce across cores
                vs_own = xtp.tile([P, HT], F32, tag="vso", bufs=1)
                for ht in range(HT):
                    scr8 = xbp.tile([P, NL], F8, tag="scr8", bufs=2)
                    nc.scalar.activation(
                        out=scr8[:],
                        in_=q8[ht // 2][:, ht % 2, :],
                        func=COPY,
                        accum_out=vs_own[:, ht : ht + 1],
                    )
                nc.sync.dma_start(out=ar_in[:], in_=vs_own[:])
                nc.gpsimd.collective_compute(
                    "AllReduce",
                    mybir.AluOpType.add,
                    replica_groups=[list(range(NCORES))],
                    ins=[ar_in[:].opt()],
                    outs=[ar_out[:].opt()],
                )

            # ---------------- Phase 2: attention ----------------
            with (
                tc.tile_pool(name="ltr", bufs=1) as ltr,
                tc.tile_pool(name="ep", bufs=6) as epool,
                tc.tile_pool(name="e8p", bufs=BLK + 2) as e8p,
                tc.tile_pool(name="vp", bufs=BLK + 2) as vp,
                tc.tile_pool(name="osb", bufs=QTP) as osb,
                tc.tile_pool(name="fin", bufs=2) as fin,
                tc.tile_pool(name="sps", bufs=3, space="PSUM") as sps,
                tc.tile_pool(name="ops", bufs=2, space="PSUM") as ops,
                tc.tile_pool(name="tvs", bufs=1, space="PSUM") as tvs,
            ):
                # Full fp8 L.T resident in DoubleRow layout: 4 tiles
                # [128, 2, 8192] (8MB), loaded per AllGather chunk.
                lt8 = [
                    ltr.tile([P, 2, N], F8, tag=f"lt{h2}", name=f"lt{h2}")
                    for h2 in range(HT2)
                ]
                for ch in range(NCH):
                    for ht in range(HT):
                        dst = lt8[ht // 2][:, ht % 2, :].rearrange(
                            "p (c g n) -> p c g n", c=NCORES, g=NCH
                        )[:, :, ch, :]
                        nc.sync.dma_start(
                            out=dst,
                            in_=ag_out[ch][:, ht].rearrange("c p n -> p c n"),
                        )

                # key-tile blocks ordered chunk-half first, so the first half
                # only depends on AllGather 0
                blocks = [
                    [c8 * RTOT + half * RT + jj for c8 in (2 * bc, 2 * bc + 1)
                     for jj in range(RT)]
                    for half in range(NCH)
                    for bc in range(NCORES // 2)
                ]

                # global column-sum row of 32*L: vrow32[0, h] = sum_k 32*L[k, h]
                vs_all = fin.tile([P, HT], F32, tag="vsa")
                nc.sync.dma_start(out=vs_all[:], in_=ar_out[:])
                vrow32 = fin.tile([1, H], F32, tag="vrow")
                for ht in range(HT):
                    vtp = tvs.tile([1, P], F32, tag="tv")
                    nc.tensor.transpose(
                        vtp[:], vs_all[:, ht : ht + 1], ident32[:]
                    )
                    nc.vector.tensor_copy(
                        out=vrow32[0:1, ht * P : (ht + 1) * P], in_=vtp[:]
                    )

                # two denominator accumulators (even/odd key tiles) halve the
                # serial DVE dependency chain; merged before the reciprocal
                csums = [
                    fin.tile([P, NL], F32, tag=f"csum{i}", name=f"csum{i}")
                    for i in range(2)
                ]
                out_sb = [
                    osb.tile([P, H], F32, tag="o", name="o") for _ in range(QTP)
                ]
                for blk, kts in enumerate(blocks):
                    e8s = []
                    v8s = []
                    for j, kt in enumerate(kts):
                        ksl = slice(kt * P, (kt + 1) * P)
                        if j % 2 == 0:
                            e8 = e8p.tile([P, 2, NL], F8, tag="e8")
                            v8 = vp.tile([P, 2, H], F8, tag="v8")
                            e8s.append(e8)
                            v8s.append(v8)
                            # direct load of the AllGathered V-natural pair
                            c, rem = kt // RTOT, kt % RTOT
                            g, jj = rem // RT, rem % RT
                            nc.gpsimd.dma_start(
                                out=v8[:],
                                in_=agv_out[g][c, jj : jj + 2].rearrange(
                                    "v p n -> p v n"
                                ),
                            )
                        eb = epool.tile([P, NL], BF, tag="e")
                        for qh in range(QH):
                            qsl = slice(qh * CHUNK, (qh + 1) * CHUNK)
                            ps = sps.tile([P, CHUNK], F32, tag="sp")
                            for h2 in range(HT2):
                                nc.tensor.matmul(
                                    ps[:],
                                    lhsT=lt8[h2][:, :, ksl],
                                    rhs=q8[h2][:, :, qsl],
                                    start=(h2 == 0),
                                    stop=(h2 == HT2 - 1),
                                    perf_mode=DR,
                                )
                            nc.scalar.activation(
                                out=eb[:, qsl], in_=ps[:], func=EXP, scale=SCALE8
                            )
                        # denominator partial (cross-partition sum at end)
                        csum = csums[j % 2]
                        if blk == 0 and j < 2:
                            nc.vector.tensor_copy(out=csum[:], in_=eb[:])
                        else:
                            nc.vector.tensor_add(out=csum[:], in0=csum[:], in1=eb[:])
                        # e' = KE*(exp(s)-1) in fp8 keeps the softmax signal
                        nc.vector.tensor_scalar(
                            out=e8[:, j % 2, :],
                            in0=eb[:],
                            scalar1=-1.0,
                            scalar2=KE,
                            op0=mybir.AluOpType.add,
                            op1=mybir.AluOpType.mult,
                        )

                    # the rank-1 correction joins block NBLK-2's chains: late
                    # enough for the AllReduce, off the critical final block
                    corr = blk == NBLK - 2
                    for qt in range(QTP):
                        po = ops.tile([P, H], F32, tag="op")
                        for j2 in range(BLK // 2):
                            for hh in range(HH):
                                nc.tensor.matmul(
                                    po[:, hh * CHUNK : (hh + 1) * CHUNK],
                                    lhsT=e8s[j2][:, :, qt * P : (qt + 1) * P],
                                    rhs=v8s[j2][:, :, hh * CHUNK : (hh + 1) * CHUNK],
                                    start=(j2 == 0),
                                    stop=(j2 == BLK // 2 - 1 and not corr),
                                    perf_mode=DR,
                                )
                        if corr:
                            # exact rank-1 term: out_unnorm*32*KE needs
                            # + KE * sum_k 32*L[k, :] added once per chain
                            for hh in range(HH):
                                nc.tensor.matmul(
                                    po[:, hh * CHUNK : (hh + 1) * CHUNK],
                                    lhsT=kerow32[0:1, :],
                                    rhs=vrow32[0:1, hh * CHUNK : (hh + 1) * CHUNK],
                                    start=False,
                                    stop=True,
                                )
                        if blk == 0:
                            nc.vector.tensor_copy(out=out_sb[qt][:], in_=po[:])
                        else:
                            nc.vector.tensor_add(
                                out=out_sb[qt][:], in0=out_sb[qt][:], in1=po[:]
                            )
                # normalize: colsum -> reciprocal row, move q to partitions.
                # V carried x32 and e' x KE (one1 = 1/(32*KE) folds both back).
                nc.vector.tensor_add(
                    out=csums[0][:], in0=csums[0][:], in1=csums[1][:]
                )
                rec_row = fin.tile([1, NL], F32, tag="rr")
                for qh in range(QH):
                    qsl = slice(qh * CHUNK, (qh + 1) * CHUNK)
                    psc = sps.tile([1, CHUNK], F32, tag="sp")
                    nc.tensor.matmul(
                        psc[:], lhsT=ones32[:], rhs=csums[0][:, qsl],
                        start=True, stop=True,
                    )
                    nc.vector.reciprocal(rec_row[0:1, qsl], psc[:])
                for qt in range(QTP):
                    ct = tvs.tile([P, 1], F32, tag="tv")
                    nc.tensor.matmul(
                        ct[:],
                        lhsT=rec_row[0:1, qt * P : (qt + 1) * P],
                        rhs=one1[0:1, 0:1],
                        start=True,
                        stop=True,
                    )
                    rec = fin.tile([P, 1], F32, tag="rec")
                    nc.vector.tensor_copy(out=rec[:], in_=ct[:])
                    nc.vector.tensor_mul(
                        out=out_sb[qt][:],
                        in0=out_sb[qt][:],
                        in1=rec[:, 0:1].to_broadcast([P, H]),
                    )
                    nc.sync.dma_start(
                        out=out[qt * P : (qt + 1) * P, :], in_=out_sb[qt][:]
                    )
    nc.finalize()
    return nc


def _prep_inputs(inputs):
    import ml_dtypes

    f8 = ml_dtypes.float8_e4m3
    ids = np.asarray(inputs["input_ids"]).astype(np.int32)
    pids = np.asarray(inputs["pos_ids"]).astype(np.int32)
    emb = (np.asarray(inputs["emb"], dtype=np.float32) * FSCALE).astype(f8)
    pemb = (np.asarray(inputs["pos_emb"], dtype=np.float32) * FSCALE).astype(f8)
    W = np.asarray(inputs["W"], dtype=np.float32)
    b = np.asarray(inputs["b"], dtype=np.float32)
    wt = np.ascontiguousarray((W.T * FSCALE).astype(f8))  # [2H, H]
    bias = np.ascontiguousarray(b.reshape(HT, P, 1))
    in_maps = []
    for i in range(NCORES):
        sl = slice(i * NL, (i + 1) * NL)
        in_maps.append(
            {
                "ids": np.ascontiguousarray(ids[sl].reshape(RTOT, P, 1)),
                "pids": np.ascontiguousarray(pids[sl].reshape(RTOT, P, 1)),
                "emb": emb,
                "pemb": pemb,
                "wt": wt,
                "bias": bias,
            }
        )
    return in_maps


def run(inputs, trace=False):
    nc = build_nc()
    in_maps = _prep_inputs(inputs)
    res = run_bass_kernel_spmd(nc, in_maps, list(range(NCORES)), trace=trace)
    out = np.concatenate([res.results[i]["out"] for i in range(NCORES)], axis=0)
    return out, res


def kernel(**inputs):
    out, _ = run(inputs, trace=False)
    return out
